# revision 1
# baseline (speedup 1.0000x reference)
"""Trainium2 Bass kernel v5: 4-pack sim (two 2-strip PSUM tiles) + e-stat PV.

Structure per core (one (batch, head) pair, N=4096, d=32, c=64):
  - q/k projections col-tiled 4x -> q4/k4 replicated on all four partition
    strips ([128, N] bf16).
  - w_out folded into values up front: vw[j,c] = sum_d v[j,d] wo[c,d] with a
    ones column producing the softmax denominator Z. PV then streams vw
    (65 cols) with exp(S^T) 128-col chunks as the STATIONARY operand, so the
    output lands directly as [i-part, c] and the epilogue is just 1/Z + DMA.
  - Sim groups cover 4 j-chunks, ALL four strips concurrently (row-packed
    K=32 matmuls; the 128x128 PE array is 16 independent 32x32 subarrays).
    Strip s holds chunks congruent to s mod 4 (kp4). The 4 outputs go to
    TWO [128, 1024] PSUM tiles (strips 0,1 / strips 2,3) so the sim pool
    stays double-buffered in 4 banks; exp runs as two [128, 1024] instrs.
  - i-blocks of 512 tokens; 4x [128,65] PSUM accumulators (one per 128-token
    chunk) accumulate over all 32 j-chunks.
  - PSUM: sim 2 banks x2 bufs + 4x o accumulators = 8 banks. NEVER two
    matmuls into sub-ranges of one bank (HW hang).
  - PV emission one group late so PE never waits on ACT exp.

All matmul operands bf16 (f32r measures 4-10x slower than bf16 on this HW).
"""

import os
import sys

for _p in ("/opt/trn_rl_repo", "/root/.axon_site/_ro/trn_rl_repo"):
    if os.path.isdir(_p) and _p not in sys.path:
        sys.path.insert(0, _p)
        break

import numpy as np

import concourse.tile as tile
from concourse import bacc, mybir

F32 = mybir.dt.float32
BF16 = mybir.dt.bfloat16
N = 4096
C = 64
D = 32
SCALE = 10.0
N_CORES = 8
IB = 512          # i-block (4 x [128, 65] o accumulators)
NIB = N // IB     # 8
NJ = N // 128     # 32 j-chunks
NG = 8            # j-groups of 4 chunks (one per strip)

REPEAT = int(os.environ.get("ATTN_REPEAT", "1"))


def build_nc(repeat=None):
    if repeat is None:
        repeat = REPEAT

    nc = bacc.Bacc(
        "TRN2",
        target_bir_lowering=False,
        debug=False,
        num_devices=N_CORES,
    )

    xb = nc.dram_tensor("xb", [C, N], F32, kind="ExternalInput").ap()
    wq = nc.dram_tensor("wq", [C, D], F32, kind="ExternalInput").ap()
    wk = nc.dram_tensor("wk", [C, D], F32, kind="ExternalInput").ap()
    wv = nc.dram_tensor("wv", [C, D], F32, kind="ExternalInput").ap()
    wo = nc.dram_tensor("wo", [D + 1, C + 1], F32, kind="ExternalInput").ap()
    out = nc.dram_tensor("out", [N, C], F32, kind="ExternalOutput").ap()

    with tile.TileContext(nc) as tc:
        with (
            tc.tile_pool(name="consts", bufs=1) as consts,
            tc.tile_pool(name="persist", bufs=1) as persist,
            tc.tile_pool(name="esb", bufs=6) as esb,
            tc.tile_pool(name="epi", bufs=2) as epi,
            tc.tile_pool(name="sim_ps", bufs=2, space="PSUM") as sim_ps,
            tc.tile_pool(name="out_ps", bufs=4, space="PSUM") as out_ps,
        ):
            wq_f = consts.tile([128, D], F32)
            wk_f = consts.tile([128, D], F32)
            wv_f = consts.tile([128, D], F32)
            wo_f = consts.tile([D + 1, C + 1], F32)
            zero_b = consts.tile([128, 1], F32)
            for w_sb, w_dram in ((wq_f, wq), (wk_f, wk), (wv_f, wv)):
                nc.sync.dma_start(out=w_sb[0:C, :], in_=w_dram)
                nc.sync.dma_start(out=w_sb[C:128, :], in_=w_dram)
            nc.sync.dma_start(out=wo_f, in_=wo)
            nc.vector.memset(zero_b, 0.0)
            wqr = consts.tile([128, D], BF16)
            wkr = consts.tile([128, D], BF16)
            wvr = consts.tile([128, D], BF16)
            wor = consts.tile([D + 1, C + 1], BF16)
            nc.vector.tensor_copy(wqr, wq_f)
            nc.vector.tensor_copy(wkr, wk_f)
            nc.vector.tensor_copy(wvr, wv_f)
            nc.vector.tensor_copy(wor, wo_f)

            x_sb = consts.tile([128, N // 2], F32)
            xr_sb = consts.tile([128, N // 2], BF16)

            def x_ap(tile_, tok0, ntok):
                half, col = divmod(tok0, N // 2)
                p0 = half * C
                return tile_[p0 : p0 + C, col : col + ntok]

            def body():
                q4 = persist.tile([128, N], BF16)
                k4 = persist.tile([128, NJ, 128], BF16)
                sq_scr = persist.tile([128, 512], F32)
                ssqp_q = persist.tile([128, N // 512], F32)
                ssqp_k = persist.tile([128, N // 512], F32)
                v_sb = persist.tile([D + 1, N], BF16)
                vw_sb = persist.tile([128, NJ, C + 1], BF16)

                # prologue copies ride on ACT (Copy is in every table set and
                # ACT measures ~4x the documented rate); DVE keeps vw + kp4
                for t in range(N // 512):
                    nc.sync.dma_start(
                        out=x_ap(x_sb, t * 512, 512),
                        in_=xb[:, t * 512 : (t + 1) * 512],
                    )
                    nc.scalar.activation(
                        x_ap(xr_sb, t * 512, 512), x_ap(x_sb, t * 512, 512),
                        mybir.ActivationFunctionType.Copy, bias=0.0,
                    )

                row = lambda t: 0 if t < 4 else C
                for t in range(N // 512):
                    sl = slice(t * 512, (t + 1) * 512)
                    c0, c1 = t * 4, (t + 1) * 4  # 128-chunk range of k4
                    xa = x_ap(xr_sb, t * 512, 512)
                    w_q = wqr[row(t) : row(t) + C, :]
                    w_k = wkr[row(t) : row(t) + C, :]
                    w_v = wvr[row(t) : row(t) + C, :]
                    ps_q = sim_ps.tile([128, 512], F32, tag="s3")
                    for s in range(4):
                        nc.tensor.matmul(
                            ps_q[32 * s : 32 * s + 32, :],
                            lhsT=w_q, rhs=xa, start=True, stop=True,
                            tile_position=(row(t), 32 * s),
                        )
                    nc.scalar.activation(
                        q4[:, sl], ps_q,
                        mybir.ActivationFunctionType.Copy, bias=0.0,
                    )
                    nc.scalar.activation(
                        sq_scr, ps_q, mybir.ActivationFunctionType.Square,
                        bias=zero_b, accum_out=ssqp_q[:, t : t + 1],
                    )
                    ps_k = sim_ps.tile([128, 512], F32, tag="s3")
                    for s in range(4):
                        nc.tensor.matmul(
                            ps_k[32 * s : 32 * s + 32, :],
                            lhsT=w_k, rhs=xa, start=True, stop=True,
                            tile_position=(row(t), 32 * s),
                        )
                    nc.scalar.activation(
                        k4[:, c0:c1, :], ps_k,
                        mybir.ActivationFunctionType.Copy, bias=0.0,
                    )
                    nc.scalar.activation(
                        sq_scr, ps_k, mybir.ActivationFunctionType.Square,
                        bias=zero_b, accum_out=ssqp_k[:, t : t + 1],
                    )
                    ps_v = sim_ps.tile([D, 512], F32, tag="s3")
                    nc.tensor.matmul(
                        ps_v, lhsT=w_v, rhs=xa, start=True, stop=True,
                    )
                    nc.scalar.activation(
                        v_sb[0:D, sl], ps_v,
                        mybir.ActivationFunctionType.Copy, bias=0.0,
                    )
                nc.vector.memset(v_sb[D : D + 1, :], 1.0)

                ssq_q = persist.tile([128, 1], F32)
                ssq_k = persist.tile([128, 1], F32)
                nc.vector.reduce_sum(
                    out=ssq_q, in_=ssqp_q, axis=mybir.AxisListType.X
                )
                nc.vector.reduce_sum(
                    out=ssq_k, in_=ssqp_k, axis=mybir.AxisListType.X
                )
                lq = persist.tile([128, 1], F32)
                lk = persist.tile([128, 1], F32)
                nc.scalar.activation(
                    lq, ssq_q, mybir.ActivationFunctionType.Ln,
                    bias=zero_b, scale=1.0 / (SCALE * SCALE),
                )
                nc.scalar.activation(
                    lk, ssq_k, mybir.ActivationFunctionType.Ln,
                    bias=zero_b,
                )
                nc.vector.tensor_add(lq, lq, lk)
                cscale = persist.tile([128, 1], F32)
                nc.scalar.activation(
                    cscale, lq, mybir.ActivationFunctionType.Exp,
                    bias=zero_b, scale=-0.5,
                )

                # kp4: strip s holds scaled chunks congruent to s mod 4.
                # Block 0 (the first j-group) is scaled first so the first
                # sims are not blocked behind the whole gather.
                kp4 = persist.tile([128, NJ // 4, 128], BF16)
                for lo, hi in ((0, 1), (1, NJ // 4)):
                    for s in range(4):
                        nc.vector.tensor_scalar_mul(
                            kp4[32 * s : 32 * s + 32, lo:hi, :],
                            k4[
                                32 * s : 32 * s + 32,
                                s + 4 * lo : s + 4 * (hi - 1) + 1 : 4,
                                :,
                            ],
                            cscale[32 * s : 32 * s + 32],
                        )

                # vw[jc] = (v_aug chunk)^T @ wo_aug -> [128, 65] per chunk.
                # Emitted AFTER kp4 so its DVE copies don't sit ahead of the
                # kp4 scaling in the in-order DVE queue and stall the sims.
                for jc in range(NJ):
                    ps_w = sim_ps.tile([128, C + 1], F32, tag="s3")
                    nc.tensor.matmul(
                        ps_w,
                        lhsT=v_sb[:, jc * 128 : (jc + 1) * 128],
                        rhs=wor,
                        start=True, stop=True,
                    )
                    nc.vector.tensor_copy(vw_sb[:, jc, :], ps_w)

                # ---- main loop ----
                o_tiles = {}
                pend = None

                def emit_pv(ib, g, e_ab):
                    for j in range(4):
                        jc = 4 * g + j
                        e_t = e_ab[j // 2]
                        for t in range(IB // 128):
                            nc.tensor.matmul(
                                o_tiles[(ib, t)],
                                lhsT=e_t[
                                    :,
                                    (j % 2) * IB + t * 128 : (j % 2) * IB
                                    + (t + 1) * 128,
                                ],
                                rhs=vw_sb[:, jc, :],
                                start=(g == 0 and j == 0),
                                stop=(g == NG - 1 and j == 3),
                            )

                def emit_epilogue(ib):
                    for t in range(IB // 128):
                        o_ps = o_tiles.pop((ib, t))
                        # fast ACT copy frees the PSUM bank for the next
                        # i-block's accumulators ~500ns earlier than waiting
                        # out the DVE recip+mul chain
                        o_sb = epi.tile([128, C + 1], F32, tag="osb")
                        nc.scalar.activation(
                            o_sb, o_ps,
                            mybir.ActivationFunctionType.Copy, bias=0.0,
                        )
                        rc = epi.tile([128, 1], F32, tag="rc")
                        nc.vector.reciprocal(rc, o_sb[:, C : C + 1])
                        f_sb = epi.tile([128, C], F32, tag="fout")
                        nc.vector.tensor_scalar_mul(f_sb, o_sb[:, 0:C], rc)
                        i0 = ib * IB + t * 128
                        nc.sync.dma_start(out=out[i0 : i0 + 128, :], in_=f_sb)

                for ib in range(NIB):
                    for t in range(IB // 128):
                        o_tiles[(ib, t)] = out_ps.tile(
                            [128, C + 1], F32, tag="o", name=f"o_ps_{ib}_{t}"
                        )
                    isl = slice(ib * IB, (ib + 1) * IB)
                    for g in range(NG):
                        e_ab = []
                        for h in range(2):
                            s_ps = sim_ps.tile(
                                [128, 2 * IB], F32, tag="s3",
                                name=f"s_ps_{ib}_{g}_{h}",
                            )
                            for j in range(2):
                                strip = 2 * h + j
                                nc.tensor.matmul(
                                    s_ps[:, j * IB : (j + 1) * IB],
                                    lhsT=kp4[
                                        32 * strip : 32 * strip + 32, g, :
                                    ],
                                    rhs=q4[32 * strip : 32 * strip + 32, isl],
                                    start=True, stop=True,
                                    tile_position=(32 * strip, 0),
                                )
                            e_t = esb.tile(
                                [128, 2 * IB], BF16, tag="e2",
                                name=f"e2_{ib}_{g}_{h}",
                            )
                            nc.scalar.activation(
                                e_t, s_ps,
                                mybir.ActivationFunctionType.Exp,
                                bias=zero_b,
                            )
                            e_ab.append(e_t)
                        if pend is not None:
                            pib, pg, pe = pend
                            emit_pv(pib, pg, pe)
                            if pg == NG - 1:
                                emit_epilogue(pib)
                        pend = (ib, g, e_ab)
                pib, pg, pe = pend
                emit_pv(pib, pg, pe)
                emit_epilogue(pib)

            for _rep in range(repeat):
                body()

    nc.compile()
    return nc


_NC_CACHE = {}


def _get_nc():
    key = REPEAT
    if key not in _NC_CACHE:
        _NC_CACHE[key] = build_nc()
    return _NC_CACHE[key]


def _make_in_maps(x, w_qkv, w_out):
    b, c, X, Y, Z = x.shape
    xr = np.ascontiguousarray(x.reshape(b, c, X * Y * Z), dtype=np.float32)
    w_qkv = np.asarray(w_qkv, dtype=np.float32)
    w_out = np.asarray(w_out, dtype=np.float32)
    in_maps = []
    for core in range(N_CORES):
        bi, h = divmod(core, 4)
        hs = slice(h * D, (h + 1) * D)
        wo_aug = np.zeros((D + 1, C + 1), dtype=np.float32)
        wo_aug[0:D, 0:C] = w_out[:, hs].T
        wo_aug[D, C] = 1.0
        in_maps.append(
            {
                "xb": xr[bi],
                "wq": np.ascontiguousarray(w_qkv[hs, :].T),
                "wk": np.ascontiguousarray(w_qkv[128 + h * D : 128 + (h + 1) * D, :].T),
                "wv": np.ascontiguousarray(w_qkv[256 + h * D : 256 + (h + 1) * D, :].T),
                "wo": wo_aug,
            }
        )
    return in_maps


def _gather(results, x_shape, b_out):
    b, c, X, Y, Z = x_shape
    n = X * Y * Z
    out = np.zeros((b, c, n), dtype=np.float32)
    for core in range(N_CORES):
        bi = core // 4
        out[bi] += results[core]["out"].T
    out += np.asarray(b_out, dtype=np.float32)[None, :, None]
    return out.reshape(b, c, X, Y, Z)


def kernel(x, w_qkv, w_out, b_out):
    from concourse.bass_utils import run_bass_kernel_spmd

    x = np.asarray(x)
    nc = _get_nc()
    in_maps = _make_in_maps(x, w_qkv, w_out)
    try:
        res = run_bass_kernel_spmd(nc, in_maps, list(range(N_CORES))).results
    except Exception:
        # A wedged NeuronCore (e.g. NRT_EXEC_UNIT_UNRECOVERABLE left over
        # from an earlier crashed process) usually recovers on re-run.
        res = run_bass_kernel_spmd(nc, in_maps, list(range(N_CORES))).results
    return _gather(res, x.shape, b_out)
# ---- appended runner/benchmark helpers (used by test.py, not the harness) ----


def _make_runner(nc, in_maps):
    """Build a reusable jitted 8-core runner with device-resident inputs.

    Mirrors bass2jax.run_bass_via_pjrt's multi-core tail, minus output
    donation, so repeated timed calls reuse on-device buffers.
    """
    import jax
    from jax.experimental.shard_map import shard_map
    from jax.sharding import Mesh, PartitionSpec

    from concourse import bass2jax, mybir as _mybir

    bass2jax.install_neuronx_cc_hook()

    partition_name = (
        nc.partition_id_tensor.name if nc.partition_id_tensor else None
    )
    in_names, out_names, out_avals, zero_outs = [], [], [], []
    for alloc in nc.m.functions[0].allocations:
        if not isinstance(alloc, _mybir.MemoryLocationSet):
            continue
        name = alloc.memorylocations[0].name
        if alloc.kind == "ExternalInput":
            if name != partition_name:
                in_names.append(name)
        elif alloc.kind == "ExternalOutput":
            out_names.append(name)
            shape = tuple(alloc.tensor_shape)
            dtype = _mybir.dt.np(alloc.dtype)
            out_avals.append(jax.core.ShapedArray(shape, dtype))
            zero_outs.append(np.zeros(shape, dtype))
    n_params = len(in_names)
    all_in_names = in_names + out_names
    if partition_name is not None:
        all_in_names = all_in_names + [partition_name]

    def _body(*args):
        operands = list(args)
        if partition_name is not None:
            operands.append(bass2jax.partition_id_tensor())
        outs = bass2jax._bass_exec_p.bind(
            *operands,
            out_avals=tuple(out_avals),
            in_names=tuple(all_in_names),
            out_names=tuple(out_names),
            lowering_input_output_aliases=(),
            sim_require_finite=True,
            sim_require_nnan=True,
            nc=nc,
        )
        return tuple(outs)

    devices = jax.devices()[:N_CORES]
    mesh = Mesh(np.asarray(devices), ("core",))
    n_outs = len(out_names)
    sharded = jax.jit(
        shard_map(
            _body,
            mesh=mesh,
            in_specs=(PartitionSpec("core"),) * (n_params + n_outs),
            out_specs=(PartitionSpec("core"),) * n_outs,
            check_rep=False,
        ),
        keep_unused=True,
    )
    sharding = jax.sharding.NamedSharding(mesh, PartitionSpec("core"))
    concat_in = [
        jax.device_put(
            np.concatenate([np.asarray(m[name]) for m in in_maps], axis=0),
            sharding,
        )
        for name in in_names
    ]
    concat_zeros = [
        jax.device_put(
            np.zeros((N_CORES * z.shape[0], *z.shape[1:]), z.dtype), sharding
        )
        for z in zero_outs
    ]

    def run():
        return sharded(*concat_in, *concat_zeros)

    return run

